# revision 1
# baseline (speedup 1.0000x reference)
"""Kent-distribution pairwise KLD loss kernel for Trainium2 (8 NeuronCores).

The [N, M] pairwise KLD matrix factors exactly as a rank-11 product
U @ V^T:

  KLD[n, m] = A[n]                                  (pred-row constant)
            + c_b[m]                                (target-row constant)
            - Ex_a[n] . (kappa_b[m] * gamma_b1[m])  (rank 3)
            + <ExxT_a[n], beta_b[m]*(g_b3 g_b3^T - g_b2 g_b2^T)>  (rank 6, sym)

so each core computes 11 features per pred row (its N-shard) and per
target row (replicated), then one skinny f32 matmul [256,11]@[11,2048].
N is sharded across the 8 cores (data parallel over predictions).

Numerics notes (validated against the jax reference to ~7e-6 absmax-rel):
 - exp(c_k - c), exp(c_kk - c) are evaluated as exact algebraic ratios
   (the ln2pi/kappa terms cancel): l1 = (k^2-k-4b^2)/D, and
   l2 = 0.5(1-e2) = 0.5(2k^3-2k^2-2sk-s)/D^2 with D = k^2-4b^2, s = 4b^2.
   This avoids the Exp activation table entirely (DVE reciprocal instead).
 - exp(c_beta - c) carries e^-kappa <= 4.5e-5 (kappa >= 10) and is dropped;
   with lambda2 == lambda3, ExxT = l2*I + (l1-l2)*g1 g1^T via orthogonality
   and the beta_a*(qa2-qa3) term vanishes.
 - Sin's HW domain is [-pi, pi]: cos(x) = sin(pi/2 - |x|), |x| on DVE.
"""

import sys

import numpy as np

sys.path.insert(0, "/opt/trn_rl_repo")

import concourse.bass as bass  # noqa: E402,F401
import concourse.mybir as mybir  # noqa: E402
import concourse.tile as tile  # noqa: E402
from concourse import bacc  # noqa: E402
from concourse.masks import make_identity  # noqa: E402

F32 = mybir.dt.float32
AF = mybir.ActivationFunctionType
ALU = mybir.AluOpType

N = 2048
M = 2048
NCORES = 8
NS = N // NCORES  # 256 pred rows per core
K = 11  # feature rank
GP = NS // 128  # pred row-groups (2)
GT = M // 128  # target row-groups (16)
G = GP + GT  # 18

PI = float(np.pi)
LN_2PI = float(np.log(2.0 * np.pi))
EPS = 1e-6
EM = float(np.exp(-1e-6))  # e^-EPS factor from the reference's den EPS


def _body(tc, pred, targ, out):
    nc = tc.nc
    with (
        tc.tile_pool(name="main", bufs=1) as pool,
        tc.tile_pool(name="tp_psum", bufs=4, space="PSUM") as tpp,
        tc.tile_pool(name="out_psum", bufs=2, space="PSUM") as opp,
    ):
        def t(shape, tag):
            return pool.tile([128, *shape], F32, name=tag, tag=tag)

        def mul(o, a, b):
            nc.vector.tensor_mul(o, a, b)

        def add(o, a, b):
            nc.vector.tensor_add(o, a, b)

        def sub(o, a, b):
            nc.vector.tensor_sub(o, a, b)

        def stt(o, in0, scalar, in1, op0, op1):
            nc.vector.scalar_tensor_tensor(o, in0, scalar, in1, op0, op1)

        # ---- load params: pred partition p holds rows 2p,2p+1; targ
        # partition p holds rows 16p..16p+15 (contiguous per-partition DMA).
        params = t([G * 5], "params")
        nc.sync.dma_start(
            out=params[:, 0 : GP * 5],
            in_=pred.rearrange("(p j) c -> p (j c)", p=128),
        )
        nc.sync.dma_start(
            out=params[:, GP * 5 : G * 5],
            in_=targ.rearrange("(p j) c -> p (j c)", p=128),
        )

        P5 = params.rearrange("p (g c) -> p c g", c=5)  # [128, 5, 18]
        kap = P5[:, 3, :]  # [128, 18] stride-5 slabs
        bet = P5[:, 4, :]

        half_pi = pool.tile([128, 1], F32, name="half_pi", tag="half_pi")
        nc.vector.memset(half_pi, PI / 2)
        eps_c = pool.tile([128, 1], F32, name="eps_c", tag="eps_c")
        nc.vector.memset(eps_c, EPS)
        # dummy Sin on a constant: hoists the trig ACT_TABLE_LOAD off the
        # input-DMA critical path (runs while the DMA is in flight)
        sin_dummy = pool.tile([128, 1], F32, name="sin_dummy", tag="sin_dummy")
        nc.scalar.activation(sin_dummy[:], half_pi[:], AF.Sin)

        # ---- sin/cos of the 3 angle columns (Sin domain is [-pi, pi])
        angles = P5[:, 0:3, :]  # [128, 3, 18]
        absv = t([3, G], "absv")
        stt(absv[:], angles, -1.0, angles, ALU.mult, ALU.max)
        sinv = t([3, G], "sinv")
        cosv = t([3, G], "cosv")
        nc.scalar.activation(sinv[:], angles, AF.Sin)
        nc.scalar.activation(cosv[:], absv[:], AF.Sin, bias=half_pi, scale=-1.0)
        se, ce = sinv[:, 0, :], cosv[:, 0, :]  # eta
        sa, ca = sinv[:, 1, :], cosv[:, 1, :]  # alpha
        sp, cp = sinv[:, 2, :], cosv[:, 2, :]  # psi

        # ---- gamma vectors for all 18 groups: gam[:, c, i, :] = gamma_{c+1}[i]
        gam = t([3, 3, G], "gam")
        nc.vector.tensor_copy(gam[:, 0, 0, :], ca)
        mul(gam[:, 0, 1, :], sa, ce)
        mul(gam[:, 0, 2, :], sa, se)
        m2 = t([G], "m2"); mul(m2, cp, ca)
        m4 = t([G], "m4"); mul(m4, sp, ca)
        spse = t([G], "spse"); mul(spse, sp, se)
        spce = t([G], "spce"); mul(spce, sp, ce)
        cpse = t([G], "cpse"); mul(cpse, cp, se)
        cpce = t([G], "cpce"); mul(cpce, cp, ce)
        m2ce = t([G], "m2ce"); mul(m2ce, m2, ce)
        m2se = t([G], "m2se"); mul(m2se, m2, se)
        m4ce = t([G], "m4ce"); mul(m4ce, m4, ce)
        m4se = t([G], "m4se"); mul(m4se, m4, se)
        # g2 = [-cp*sa, m2*ce - sp*se, m2*se + sp*ce]
        stt(gam[:, 1, 0, :], cp, -1.0, sa, ALU.mult, ALU.mult)
        sub(gam[:, 1, 1, :], m2ce, spse)
        add(gam[:, 1, 2, :], m2se, spce)
        # g3 = [sp*sa, -(m4*ce + cp*se), cp*ce - m4*se]
        mul(gam[:, 2, 0, :], sp, sa)
        stt(gam[:, 2, 1, :], m4ce, -1.0, cpse, ALU.mult, ALU.subtract)
        sub(gam[:, 2, 2, :], cpce, m4se)

        # ---- pair products p_c_e for e in [00,11,22,01,02,12]
        prod = t([3, 6, G], "prod")
        mul(prod[:, :, 0:3, :], gam[:], gam[:])
        mul(
            prod[:, :, 3:5, :],
            gam[:, :, 0:1, :].broadcast_to([128, 3, 2, G]),
            gam[:, :, 1:3, :],
        )
        mul(prod[:, :, 5, :], gam[:, :, 1, :], gam[:, :, 2, :])

        # ---- kappa/beta shared slabs + c = ln2pi + k - 0.5 ln((k-2b)(k+2b)+EPS)
        b2 = t([G], "b2"); add(b2, bet, bet)  # 2*beta
        km = t([G], "km"); sub(km, kap, b2)
        kp = t([G], "kp"); add(kp, kap, b2)
        LNIN = t([G], "LNIN")
        mul(LNIN[:], km, kp)
        LNOUT = t([G], "LNOUT")
        nc.scalar.activation(LNOUT[:], LNIN[:], AF.Ln, bias=eps_c)
        lnprod_p = LNOUT[:, 0:GP]
        lnprod_t = LNOUT[:, GP:G]

        # ---- target features VF [128, 11, 16]
        VF = t([K, GT], "VF")
        nc.vector.memset(VF[:, 0, :], 1.0)
        cb1 = t([GT], "cb1")
        stt(cb1, lnprod_t, -0.5, kap[:, GP:G], ALU.mult, ALU.add)
        nc.vector.tensor_scalar_add(VF[:, 1, :], cb1, LN_2PI)
        negk = t([GT], "negk")
        nc.vector.tensor_scalar_mul(negk, kap[:, GP:G], -1.0)
        mul(
            VF[:, 2:5, :],
            gam[:, 0, :, GP:G],
            negk.unsqueeze(1).broadcast_to([128, 3, GT]),
        )
        dV = t([6, GT], "dV")
        sub(dV, prod[:, 2, :, GP:G], prod[:, 1, :, GP:G])
        mul(
            VF[:, 5:8, :],
            dV[:, 0:3, :],
            bet[:, GP:G].unsqueeze(1).broadcast_to([128, 3, GT]),
        )
        mul(
            VF[:, 8:11, :],
            dV[:, 3:6, :],
            b2[:, GP:G].unsqueeze(1).broadcast_to([128, 3, GT]),
        )

        # ---- transpose targets to group-major VT [11, 2048] (col = 128j + p)
        ident = t([128], "ident")
        make_identity(nc, ident)
        VT = pool.tile([K, M], F32, name="VT", tag="VT")
        for q in range(4):
            vtp = tpp.tile([K, 512], F32, name="vtp", tag="vtp", bufs=3)
            for jj in range(4):
                j = q * 4 + jj
                nc.tensor.transpose(
                    vtp[:, jj * 128 : (jj + 1) * 128], VF[:, :, j], ident[:]
                )
            nc.scalar.copy(VT[:, q * 512 : (q + 1) * 512], vtp[:])

        # ---- pred features UF [128, 11, 2] (exp-free lambda chain)
        kap_p = kap[:, 0:GP]
        x2 = t([GP], "x2"); mul(x2, kap_p, kap_p)
        kap2 = t([GP], "kap2"); add(kap2, kap_p, kap_p)
        s4 = t([GP], "s4"); mul(s4, b2[:, 0:GP], b2[:, 0:GP])  # s = 4 b^2
        D = t([GP], "D"); sub(D, x2, s4)
        r = t([GP], "r"); nc.vector.reciprocal(r, D)
        r2 = t([GP], "r2"); mul(r2, r, r)
        tneg = t([GP], "tneg"); sub(tneg, x2, kap_p)
        neg = t([GP], "neg"); sub(neg, tneg, s4)  # k^2 - k - s
        l1 = t([GP], "l1")
        stt(l1, neg, EM, r, ALU.mult, ALU.mult)  # l1 = (neg*EM)*r
        # l2 = 0.5 * (2k^2(k-1) - s(2k+1)) / D^2
        t2_ = t([GP], "t2_")
        stt(t2_, kap_p, -1.0, x2, ALU.add, ALU.mult)  # x^2 (k-1)
        t3_ = t([GP], "t3_"); add(t3_, t2_, t2_)
        t5_ = t([GP], "t5_")
        stt(t5_, kap2, 1.0, s4, ALU.add, ALU.mult)  # s (2k+1)
        Q = t([GP], "Q"); sub(Q, t3_, t5_)
        l2 = t([GP], "l2")
        stt(l2, Q, 0.5, r2, ALU.mult, ALU.mult)
        dE = t([GP], "dE"); sub(dE, l1, l2)

        UF = t([K, GP], "UF")
        nc.vector.memset(UF[:, 1, :], 1.0)
        # Ex_a = l1 * g1
        mul(
            UF[:, 2:5, :],
            gam[:, 0, :, 0:GP],
            l1.unsqueeze(1).broadcast_to([128, 3, GP]),
        )
        # ExxT = l2 I + (l1 - l2) g1 g1^T
        edt = t([3, GP], "edt")
        mul(edt, prod[:, 0, 0:3, 0:GP], dE.unsqueeze(1).broadcast_to([128, 3, GP]))
        add(UF[:, 5:8, :], edt, l2.unsqueeze(1).broadcast_to([128, 3, GP]))
        mul(
            UF[:, 8:11, :],
            prod[:, 0, 3:6, 0:GP],
            dE.unsqueeze(1).broadcast_to([128, 3, GP]),
        )
        # A = (0.5 lnprod - k) + k l1 |g1|^2 - ln2pi
        ts1 = t([GP], "ts1"); add(ts1, prod[:, 0, 0, 0:GP], prod[:, 0, 1, 0:GP])
        ts2 = t([GP], "ts2"); add(ts2, ts1, prod[:, 0, 2, 0:GP])
        sdot = t([GP], "sdot"); mul(sdot, l1, ts2)
        kadot = t([GP], "kadot"); mul(kadot, kap_p, sdot)
        a1 = t([GP], "a1")
        stt(a1, lnprod_p, 0.5, kap_p, ALU.mult, ALU.subtract)
        a2 = t([GP], "a2"); add(a2, a1, kadot)
        nc.vector.tensor_scalar_add(UF[:, 0, :], a2, -LN_2PI)

        # ---- transpose preds to interleaved UT [11, 256] (col = pred row);
        # the stationary matmul operand must be a single free dim, so the
        # interleave happens in this copy (dest stride GP)
        UT = pool.tile([K, NS], F32, name="UT", tag="UT")
        utp = tpp.tile([K, GP * 128], F32, name="utp", tag="utp", bufs=1)
        for j in range(GP):
            nc.tensor.transpose(utp[:, j * 128 : (j + 1) * 128], UF[:, :, j], ident[:])
        nc.scalar.copy(
            UT.rearrange("k (p j) -> k j p", j=GP),
            utp.rearrange("k (j p) -> k j p", p=128),
        )

        # ---- main matmuls: VT stays group-major; the moving operand reads it
        # with a [p, j] strided AP so output columns land in natural m order
        VTv = VT.rearrange("k (j p) -> k p j", p=128)  # [11, 128, 16]
        outv = out.rearrange("(t p) m -> p t m", p=128)  # row = 128 t + p
        for ti in range(GP):
            for h in range(2):
                ops = opp.tile([128, 1024], F32, name="ops", tag="ops")
                for cc in range(2):
                    c = 2 * h + cc
                    nc.tensor.matmul(
                        ops[:, cc * 512 : (cc + 1) * 512],
                        UT[:, 128 * ti : 128 * (ti + 1)],
                        VTv[:, 32 * c : 32 * (c + 1), :],
                        start=True,
                        stop=True,
                    )
                out_sb = pool.tile(
                    [128, 1024], F32, name="out_sb", tag="out_sb", bufs=4
                )
                if (ti + h) % 2 == 0:
                    nc.vector.tensor_copy(out_sb[:], ops[:])
                else:
                    nc.scalar.copy(out_sb[:], ops[:])
                nc.sync.dma_start(
                    out=outv[:, ti, h * 1024 : (h + 1) * 1024], in_=out_sb[:]
                )


def build():
    nc = bacc.Bacc()
    pred = nc.dram_tensor("pred", [NS, 5], F32, kind="ExternalInput")
    targ = nc.dram_tensor("targ", [M, 5], F32, kind="ExternalInput")
    out = nc.dram_tensor("out", [NS, M], F32, kind="ExternalOutput")
    with tile.TileContext(nc) as tc:
        _body(tc, pred[:], targ[:], out[:])
    nc.finalize()
    return nc


_NC_CACHE = None


def _get_nc():
    global _NC_CACHE
    if _NC_CACHE is None:
        _NC_CACHE = build()
    return _NC_CACHE


def kernel(kent_pred, kent_target, trace=False, tmpdir=None):
    from concourse.bass_utils import run_bass_kernel_spmd

    nc = _get_nc()
    kent_pred = np.ascontiguousarray(np.asarray(kent_pred, dtype=np.float32))
    kent_target = np.ascontiguousarray(np.asarray(kent_target, dtype=np.float32))
    in_maps = [
        {"pred": kent_pred[i * NS : (i + 1) * NS], "targ": kent_target}
        for i in range(NCORES)
    ]
    res = run_bass_kernel_spmd(
        nc, in_maps, core_ids=list(range(NCORES)), trace=trace, tmpdir=tmpdir
    )
    out = np.concatenate([r["out"] for r in res.results], axis=0)
    if trace:
        kernel.last_results = res
    return out



# revision 4
# speedup vs baseline: 1.4104x; 1.4104x over previous
"""Kent-distribution pairwise KLD loss kernel for Trainium2 (8 NeuronCores).

v2: bf16 single-pass matmul with exact hi/lo (split-float) features.

The [N, M] pairwise KLD matrix factors exactly as a rank-11 product
U @ V^T (see baseline notes).  v2 reformulates the fp32 matmul (which
runs at 4 cyc/col x 2 passes on the PE) as ONE bf16 matmul (1 cyc/col)
with contraction K=44: each fp32 feature x is represented as
bf16 hi + bf16 lo (lo = bf16(x - hi), so hi+lo == x to ~2^-17 rel) and

  U44 = [Uh; Ul; Uh; Ul],  V44 = [Vh; Vh; Vl; Vl]
  sum_k U44[k] V44[k] = sum_f (Uh+Ul)(Vh+Vl) = U . V  (all cross terms)

Numerics (vs jax reference):
 - l1 = (k^2-k-4b^2)/D, l2 = 0.5(2k^2(k-1)-s(2k+1))/D^2, D = k^2-4b^2,
   s = 4b^2 (exact algebraic ratios; exp(-EPS) factor ~1e-6 dropped).
 - |gamma1|^2 == 1 exactly => kappa_a.Ex_a = k*l1 (3 ops saved).
 - l2 * sum(dVdiag) == 0 (|g_b2|=|g_b3|=1) => l2 dropped from UF[5:8].
 - LN_2PI cancels between c_b and -c_a => dropped from both features.
 - G3 := -gamma3 throughout (sign-insensitive: only quadratic uses).
 - Sin HW domain is [-pi,pi]: cos(x) = sin(pi/2 - |x|).

Elementwise chain is packed via strided APs over one workspace tile W
(slot axis x 18 group columns); target groups live in columns 2:18,
pred groups in columns 0:2 (slots can overlap when columns differ).
"""

import sys

import numpy as np

sys.path.insert(0, "/opt/trn_rl_repo")

import concourse.bass as bass  # noqa: E402,F401
import concourse.mybir as mybir  # noqa: E402
import concourse.tile as tile  # noqa: E402
from concourse import bacc  # noqa: E402
from concourse.masks import make_identity  # noqa: E402

F32 = mybir.dt.float32
BF16 = mybir.dt.bfloat16
AF = mybir.ActivationFunctionType
ALU = mybir.AluOpType

N = 2048
M = 2048
NCORES = 8
NS = N // NCORES  # 256 pred rows per core
K = 11  # fp32 feature rank
K4 = 4 * K  # bf16 hi/lo doubled contraction
GP = NS // 128  # pred row-groups (2)
GT = M // 128  # target row-groups (16)
G = GP + GT  # 18

PI = float(np.pi)
EPS = 1e-6


def _body(tc, pred, targ, out):
    nc = tc.nc
    with (
        tc.tile_pool(name="main", bufs=1) as pool,
        tc.tile_pool(name="vt_psum", bufs=4, space="PSUM") as vpp,
        tc.tile_pool(name="ut_psum", bufs=1, space="PSUM") as upp,
        tc.tile_pool(name="out_psum", bufs=3, space="PSUM") as opp,
    ):
        def t(shape, tag, dtype=F32):
            return pool.tile([128, *shape], dtype, name=tag, tag=tag)

        dve = nc.vector
        act = nc.scalar
        gps = nc.gpsimd

        # ---- input DMAs first (exec time starts at the first non-seq
        # instruction, so issue these before any memset): pred partition p
        # holds rows 2p,2p+1; targ partition p holds rows 16p..16p+15.
        params = t([G * 5], "params")
        nc.sync.dma_start(
            out=params[:, 0 : GP * 5],
            in_=pred.rearrange("(p j) c -> p (j c)", p=128),
        )
        act.dma_start(
            out=params[:, GP * 5 : G * 5],
            in_=targ.rearrange("(p j) c -> p (j c)", p=128),
        )

        P5 = params.rearrange("p (g c) -> p c g", c=5)  # [128, 5, 18]
        kap = P5[:, 3, :]  # [128, 18] stride-5
        bet = P5[:, 4, :]
        kap_p, bet_p = kap[:, 0:GP], bet[:, 0:GP]
        kap_t, bet_t = kap[:, GP:G], bet[:, GP:G]

        # ---- constants (issued after the DMAs; they overlap the transfer)
        half_pi = pool.tile([128, 1], F32, name="half_pi", tag="half_pi")
        gps.memset(half_pi, PI / 2)
        eps_c = pool.tile([128, 1], F32, name="eps_c", tag="eps_c")
        gps.memset(eps_c, EPS)
        one_c = pool.tile([128, 1], F32, name="one_c", tag="one_c")
        gps.memset(one_c, 1.0)
        half_c = pool.tile([128, 1], F32, name="half_c", tag="half_c")
        gps.memset(half_c, 0.5)
        ident = pool.tile([128, 128], BF16, name="ident", tag="ident")
        make_identity(nc, ident)

        # dummy Sin on a constant hoists the trig ACT_TABLE_LOAD so it runs
        # while the input DMA is in flight
        sin_dummy = pool.tile([128, 1], F32, name="sin_dummy", tag="sin_dummy")
        act.activation(sin_dummy[:], half_pi[:], AF.Sin)

        # ---- workspace W: slot axis x 18 group columns.
        # 0 ce, 1 ca, 2 cp | 3 se, 4 sa, 5 sp | 6 g1x, 7 g1y, 8 g1z |
        # 9 m2, 10 m4 | 11 spce, 12 spse, 13 cpce, 14 cpse |
        # 15 m2ce, 16 m2se, 17 m4ce, 18 m4se |
        # 19 g2x, 20 g2y, 21 g2z | 22 G3x, 23 G3y, 24 G3z |
        # 25:31 squares [g2,G3] (V cols) / 25:28 p1diag (U cols) |
        # 31 g2xy, 32 g2xz, 33 g2yz, 34 G3xy, 35 G3xz, 36 G3yz (V) /
        #   28 p1xy, 29 p1xz, 30 p1yz (U cols) |
        # 37:40 dVdiag, 40:43 dVoff | 43 km, 44 kp, 45 LNIN, 46 lnprod |
        # 47:58 VF (V cols; 47 is the const-1 feature) | 58:61 absv | 61 b2
        S = 62
        W = t([S, G], "W")
        gps.memset(W[:, 47, :], 1.0)  # VF0 = 1

        angles = P5[:, 0:3, :]
        absv = W[:, 58:61, :]
        dve.scalar_tensor_tensor(absv, angles, -1.0, angles, ALU.mult, ALU.max)
        # sinv -> slots 3:6 [se, sa, sp]; cosv -> 0:3 [ce, ca, cp]
        act.activation(W[:, 3:6, :], angles, AF.Sin)
        act.activation(W[:, 0:3, :], absv, AF.Sin, bias=half_pi, scale=-1.0)

        # ---- kappa/beta shared (Pool, only needs the DMA).
        # Pool has no TensorScalarPtr, so everything is tensor_tensor with
        # const tiles. b2 = 2*beta in W[61].
        gps.tensor_add(W[:, 61, :], bet, bet)
        gps.tensor_sub(W[:, 43, :], kap, W[:, 61, :])  # km
        gps.tensor_add(W[:, 44, :], kap, W[:, 61, :])  # kp
        gps.tensor_mul(W[:, 45, :], W[:, 43, :], W[:, 44, :])
        # lnprod = ln((k-2b)(k+2b) + EPS)  [ACT, ln table auto-loads first]
        act.activation(W[:, 46, :], W[:, 45, :], AF.Ln, bias=eps_c)

        # ---- U-side rational chain (Pool; free size 2) ----
        # l2 = 0.5(2k^2(k-1) - s(2k+1))/D^2 = ((k-1)k^2 - ks - s/2)/D^2
        U = t([20, GP], "U")
        b2p = W[:, 61, 0:GP]
        one_b = one_c.broadcast_to([128, GP])
        half_b = half_c.broadcast_to([128, GP])

        def u(i):
            return U[:, i, :]

        gps.tensor_mul(u(0), kap_p, kap_p)  # x2 = k^2
        gps.tensor_mul(u(1), b2p, b2p)  # s = 4b^2
        gps.tensor_sub(u(2), u(0), u(1))  # D
        dve.reciprocal(u(3), u(2))  # rec = 1/D
        gps.tensor_sub(u(4), u(0), kap_p)  # n1 = k^2-k
        gps.tensor_sub(u(5), u(4), u(1))  # n2 = k^2-k-s
        gps.tensor_mul(u(6), u(5), u(3))  # l1
        gps.tensor_sub(u(7), kap_p, one_b)  # k-1
        gps.tensor_mul(u(8), u(7), u(0))  # t2 = (k-1)k^2
        gps.tensor_mul(u(9), kap_p, u(1))  # ks
        gps.tensor_mul(u(10), bet_p, b2p)  # s/2 = 2b^2
        gps.tensor_sub(u(11), u(8), u(9))
        gps.tensor_sub(u(12), u(11), u(10))  # Qh = t2 - ks - s/2
        gps.tensor_mul(u(13), u(3), u(3))  # rec^2
        gps.tensor_mul(u(14), u(12), u(13))  # l2
        gps.tensor_sub(u(15), u(6), u(14))  # dE = l1 - l2
        gps.tensor_mul(u(16), kap_p, u(6))  # kadot = k*l1 (|g1|^2 == 1)

        # ---- gamma chain (DVE), packed via strided slot APs ----
        def rep_outer(ap, n):  # [128, a, g] -> [128, a, n, g]
            return ap.unsqueeze(2).broadcast_to([128, ap.shape[1], n, ap.shape[2]])

        def rep_inner(ap, n):  # [128, b, g] -> [128, n, b, g]
            return ap.unsqueeze(1).broadcast_to([128, n, ap.shape[1], ap.shape[2]])

        cpsp = W[:, 2:6:3, :]  # [cp, sp]
        cese = W[:, 0:4:3, :]  # [ce, se]
        sa_b = W[:, 4:5, :].broadcast_to([128, 2, G])
        # m2 = cp*ca, m4 = sp*ca
        dve.tensor_mul(W[:, 9:11, :], cpsp, W[:, 1:2, :].broadcast_to([128, 2, G]))
        # g1x = ca (copy), [g1y, g1z] = sa * [ce, se]
        dve.tensor_copy(W[:, 6:7, :], W[:, 1:2, :])
        dve.tensor_mul(W[:, 7:9, :], sa_b, cese)
        # aux1: [spce, spse, cpce, cpse] = [sp,sp,cp,cp]*[ce,se,ce,se]
        aux1_out = W[:, 11:15, :].rearrange("p (a b) g -> p a b g", a=2)
        dve.tensor_mul(aux1_out, rep_outer(W[:, 5:1:-3, :], 2), rep_inner(cese, 2))
        # aux2: [m2ce, m2se, m4ce, m4se] = [m2,m2,m4,m4]*[ce,se,ce,se]
        aux2_out = W[:, 15:19, :].rearrange("p (a b) g -> p a b g", a=2)
        dve.tensor_mul(aux2_out, rep_outer(W[:, 9:11, :], 2), rep_inner(cese, 2))
        # [g2x, G3x] = -[cp, sp]*sa   (G3 = -gamma3; sign-insensitive)
        dve.scalar_tensor_tensor(W[:, 19:23:3, :], cpsp, -1.0, sa_b, ALU.mult, ALU.mult)
        # [g2y, G3z] = [m2ce, m4se] - [spse, cpce]
        dve.tensor_sub(W[:, 20:25:4, :], W[:, 15:19:3, :], W[:, 12:14, :])
        # [g2z, G3y] = [m2se, m4ce] + [spce, cpse]
        dve.tensor_add(W[:, 21:24:2, :], W[:, 16:18, :], W[:, 11:15:3, :])

        # ---- V pair products (target columns 2:18) ----
        g6t = W[:, 19:25, GP:G]
        dve.tensor_mul(W[:, 25:31, GP:G], g6t, g6t)  # squares
        offa_in1 = W[:, 19:25, GP:G].rearrange("p (a b) g -> p a b g", a=2)[:, :, 1:3, :]
        offa_out = W[:, 31:37, GP:G].rearrange("p (a b) g -> p a b g", a=2)[:, :, 0:2, :]
        dve.tensor_mul(offa_out, rep_outer(W[:, 19:25:3, GP:G], 2), offa_in1)
        dve.tensor_mul(W[:, 33:39:3, GP:G], W[:, 20:26:3, GP:G], W[:, 21:27:3, GP:G])
        dve.tensor_sub(W[:, 37:40, GP:G], W[:, 28:31, GP:G], W[:, 25:28, GP:G])
        dve.tensor_sub(W[:, 40:43, GP:G], W[:, 34:37, GP:G], W[:, 31:34, GP:G])

        # ---- U pair products p1 = g1 (x) g1 (pred columns 0:2, Pool) ----
        g1p = W[:, 6:9, 0:GP]
        gps.tensor_mul(W[:, 25:28, 0:GP], g1p, g1p)
        gps.tensor_mul(
            W[:, 28:30, 0:GP],
            W[:, 6:7, 0:GP].broadcast_to([128, 2, GP]),
            W[:, 7:9, 0:GP],
        )
        gps.tensor_mul(W[:, 30, 0:GP], W[:, 7, 0:GP], W[:, 8, 0:GP])

        # ---- V features (fp32) into W[47:58] ----
        kt3 = kap_t.unsqueeze(1).broadcast_to([128, 3, GT])
        bt3 = bet_t.unsqueeze(1).broadcast_to([128, 3, GT])
        # VF1 = k_b - 0.5*lnprod_b   (LN_2PI cancels against -c_a)
        dve.scalar_tensor_tensor(
            W[:, 48, GP:G], W[:, 46, GP:G], -0.5, kap_t, ALU.mult, ALU.add
        )
        # VF[2:5] = -k_b * g_b1
        dve.scalar_tensor_tensor(
            W[:, 49:52, GP:G], W[:, 6:9, GP:G], -1.0, kt3, ALU.mult, ALU.mult
        )
        dve.tensor_mul(W[:, 52:55, GP:G], W[:, 37:40, GP:G], bt3)  # beta*dVdiag
        dve.scalar_tensor_tensor(
            W[:, 55:58, GP:G], W[:, 40:43, GP:G], 2.0, bt3, ALU.mult, ALU.mult
        )

        # ---- U features (fp32) ----
        UF = t([K, GP], "UF")
        gps.memset(UF[:, 1, :], 1.0)
        l1b = U[:, 6:7, :].broadcast_to([128, 3, GP])
        deb = U[:, 15:16, :].broadcast_to([128, 3, GP])
        de2 = U[:, 15:16, :].broadcast_to([128, 2, GP])
        gps.tensor_mul(UF[:, 2:5, :], g1p, l1b)  # Ex = l1*g1
        gps.tensor_mul(UF[:, 5:8, :], W[:, 25:28, 0:GP], deb)  # dE*p1diag
        gps.tensor_mul(UF[:, 8:10, :], W[:, 28:30, 0:GP], de2)
        gps.tensor_mul(UF[:, 10, :], W[:, 30, 0:GP], U[:, 15, :])
        # A = 0.5*lnprod - k + k*l1   (const LN_2PI dropped)
        gps.tensor_mul(u(17), W[:, 46, 0:GP], half_b)
        gps.tensor_sub(u(18), u(17), kap_p)
        gps.tensor_add(UF[:, 0, :], u(18), u(16))

        # ---- hi/lo split to bf16: V44 = [Vh; Vh; Vl; Vl], U44 = [Uh; Ul; Uh; Ul]
        # so sum_k U44[k]*V44[k] = sum_f (Uh+Ul)(Vh+Vl) = exact U.V
        VH = t([K4, GT], "VH", BF16)
        UH = t([K4, GP], "UH", BF16)
        VF = W[:, 47:58, GP:G]
        # V hi: slots [0:11] and [11:22] (written twice via a rep-2 out AP)
        vh_hi = VH[:, 0 : 2 * K, :].rearrange("p (r k) g -> p r k g", r=2)
        vh_lo = VH[:, 2 * K : 4 * K, :].rearrange("p (r k) g -> p r k g", r=2)
        act.copy(vh_hi, rep_inner(VF, 2))
        dve.tensor_sub(vh_lo, rep_inner(VF, 2), rep_inner(VH[:, 0:K, :], 2))
        # U hi: slots [0:11] and [22:33]; lo: [11:22] and [33:44]
        uh2 = UH.rearrange("p (r k) g -> p r k g", r=2)
        gps.tensor_copy(uh2[:, :, 0:K, :], rep_inner(UF[:], 2))
        gps.tensor_sub(
            uh2[:, :, K : 2 * K, :], rep_inner(UF[:], 2), rep_inner(UH[:, 0:K, :], 2)
        )

        # ---- transposes (PE, bf16): U then V ----
        utp = upp.tile([K4, 1024], BF16, name="utp", tag="utp")
        for j in range(GP):
            nc.tensor.transpose(utp[:, j * 128 : (j + 1) * 128], UH[:, :, j], ident[:])
        UT = pool.tile([K4, NS], BF16, name="UT", tag="UT")
        # interleave: UT col (= pred row) r = 2p + j
        act.copy(
            UT.rearrange("k (p j) -> k j p", j=GP),
            utp[:, 0 : GP * 128].rearrange("k (j p) -> k j p", p=128),
        )

        VT = pool.tile([K4, M], BF16, name="VT", tag="VT")
        for q in range(4):
            vtp = vpp.tile([K4, 1024], BF16, name="vtp", tag="vtp")
            for jj in range(4):
                j = q * 4 + jj
                nc.tensor.transpose(
                    vtp[:, jj * 128 : (jj + 1) * 128], VH[:, :, j], ident[:]
                )
            if q % 2 == 0:
                dve.tensor_copy(VT[:, q * 512 : (q + 1) * 512], vtp[:, 0:512])
            else:
                act.copy(VT[:, q * 512 : (q + 1) * 512], vtp[:, 0:512])

        # ---- main matmuls (bf16, K=44): 8 x [44,128] @ [44,512] ----
        VTv = VT.rearrange("k (j p) -> k p j", p=128)  # col m = 16p + j
        outv = out.rearrange("(t p) m -> p t m", p=128)  # row = 128 t + p
        ci = 0
        for ti in range(GP):
            for c in range(4):
                ops = opp.tile([128, 512], F32, name="ops", tag="ops")
                nc.tensor.matmul(
                    ops[:],
                    UT[:, 128 * ti : 128 * (ti + 1)],
                    VTv[:, 32 * c : 32 * (c + 1), :],
                    start=True,
                    stop=True,
                )
                out_sb = pool.tile(
                    [128, 512], F32, name="out_sb", tag="out_sb", bufs=4
                )
                if ci % 2 == 0:
                    dve.tensor_copy(out_sb[:], ops[:])
                else:
                    act.copy(out_sb[:], ops[:])
                ring = nc.sync if ci % 2 == 0 else act
                ring.dma_start(
                    out=outv[:, ti, 512 * c : 512 * (c + 1)], in_=out_sb[:]
                )
                ci += 1


def build():
    nc = bacc.Bacc()
    pred = nc.dram_tensor("pred", [NS, 5], F32, kind="ExternalInput")
    targ = nc.dram_tensor("targ", [M, 5], F32, kind="ExternalInput")
    out = nc.dram_tensor("out", [NS, M], F32, kind="ExternalOutput")
    with tile.TileContext(nc) as tc:
        _body(tc, pred[:], targ[:], out[:])
    nc.finalize()
    return nc


_NC_CACHE = None


def _get_nc():
    global _NC_CACHE
    if _NC_CACHE is None:
        _NC_CACHE = build()
    return _NC_CACHE


def kernel(kent_pred, kent_target, trace=False, tmpdir=None):
    from concourse.bass_utils import run_bass_kernel_spmd

    nc = _get_nc()
    kent_pred = np.ascontiguousarray(np.asarray(kent_pred, dtype=np.float32))
    kent_target = np.ascontiguousarray(np.asarray(kent_target, dtype=np.float32))
    in_maps = [
        {"pred": kent_pred[i * NS : (i + 1) * NS], "targ": kent_target}
        for i in range(NCORES)
    ]
    res = run_bass_kernel_spmd(
        nc, in_maps, core_ids=list(range(NCORES)), trace=trace, tmpdir=tmpdir
    )
    out = np.concatenate([r["out"] for r in res.results], axis=0)
    if trace:
        kernel.last_results = res
    return out
